# revision 1
# baseline (speedup 1.0000x reference)
"""Causal multi-head attention forward on 8 Trainium2 NeuronCores.

Problem: nn_CoreAttention (SQ=SK=2048, B=2, NP=16 heads, HN=128, fp32).

Sharding: the 32 (batch, head) pairs are split 4 per core (tensor-parallel
over heads, data-parallel over batch). No collectives needed.

Per (b, n) pair the kernel computes, in transposed score orientation:
    scoresT[sk, sq] = (K Q^T) / sqrt(HN)      (PE matmul, hn contracted)
    expT = exp(scoresT + additive_mask)       (ScalarE, fused scale, fp16 out)
    ctx_aug[sq, hn+1] = expT^T @ [V | 1]      (PE matmul, sk contracted;
                                               col hn holds the softmax denom)
    ctx = ctx_aug[:, :hn] * 1/ctx_aug[:, hn]  (DVE reciprocal + scale)

The block schedule (which 128x128 score blocks are skipped / masked) is
derived from the actual attention_mask at build time, so any mask pattern
produces a correct (if differently-sized) kernel. The causal mask gives the
standard lower-triangular schedule with one unique triangular additive tile.
"""

import math
import numpy as np
from contextlib import ExitStack

import concourse.bacc as bacc
import concourse.tile as tile
from concourse import mybir

SQ, SK, B, NP, HN = 2048, 2048, 2, 16, 128
N_CORES = 8
SLOTS_PER_CORE = 4  # (b, n) pairs per core
P = 128             # partition dim / block size
CHUNK = 256         # sq chunk width for QK matmuls (fp16/fp32r full rate)
import os
GROUP = int(os.environ.get("ATT_GROUP", "4"))
SC_BUFS = int(os.environ.get("ATT_SC_BUFS", "3"))
CX_BUFS = int(os.environ.get("ATT_CX_BUFS", "2"))
E_BUFS = int(os.environ.get("ATT_E_BUFS", "6"))
N_SQ_TILES = SQ // P        # 16
N_SK_TILES = SK // P        # 16
N_CHUNKS = SQ // CHUNK      # 8
NEG = -60000.0              # additive mask value; exp -> exactly 0

QK_MODE = os.environ.get("ATT_QK_MODE", "fp32r")  # "fp32r" | "fp16" | "bf16x3"

F32 = mybir.dt.float32
F32R = mybir.dt.float32r
F16 = mybir.dt.float16
BF16 = mybir.dt.bfloat16

SKIP, FULL, PARTIAL = 0, 1, 2


def _block_schedule(mask_b: np.ndarray):
    """Classify each 128x128 (sk_tile j, sq_tile i) block of one batch's mask.

    Returns (status[j][i], tiles) where tiles maps uid -> additive fp32
    [128(sk), 128(sq)] tile (transposed into scoresT orientation).
    """
    m4 = mask_b.reshape(N_SQ_TILES, P, N_SK_TILES, P)
    alls = m4.all(axis=(1, 3))  # [i, j]
    anys = m4.any(axis=(1, 3))
    status = np.zeros((N_SK_TILES, N_SQ_TILES), dtype=np.int64)
    tiles: dict[bytes, int] = {}
    uniq: list[np.ndarray] = []
    uid_of: dict[tuple[int, int], int] = {}
    for j in range(N_SK_TILES):
        for i in range(N_SQ_TILES):
            if alls[i, j]:
                status[j, i] = SKIP
            elif not anys[i, j]:
                status[j, i] = FULL
            else:
                status[j, i] = PARTIAL
                t = np.where(m4[i, :, j, :].T, np.float32(NEG), np.float32(0.0))
                key = t.tobytes()
                if key not in tiles:
                    tiles[key] = len(uniq)
                    uniq.append(t)
                uid_of[(j, i)] = tiles[key]
    return status, uniq, uid_of


def _build_program(schedules, n_mask_tiles):
    """Build the SPMD bass program. schedules[slot] = (status, uid_of)."""
    nc = bacc.Bacc()

    qT_d = nc.declare_dram_parameter("qT", [SLOTS_PER_CORE, P, SQ], F32, isOutput=False)
    kT_d = nc.declare_dram_parameter("kT", [SLOTS_PER_CORE, P, SK], F32, isOutput=False)
    v_d = nc.declare_dram_parameter(
        "v_aug", [SLOTS_PER_CORE, P, N_SK_TILES, HN + 1], F16, isOutput=False
    )
    mt_d = None
    if n_mask_tiles:
        mt_d = nc.declare_dram_parameter(
            "mask_tiles", [P, n_mask_tiles * P], F32, isOutput=False
        )
    out_d = nc.declare_dram_parameter(
        "out", [SLOTS_PER_CORE, N_SQ_TILES, P, HN], F32, isOutput=True
    )

    inv_norm = 1.0 / math.sqrt(HN)

    with tile.TileContext(nc) as tc, ExitStack() as ctx:
        qk_pool = ctx.enter_context(tc.tile_pool(name="qk", bufs=2))
        qkr_pool = ctx.enter_context(tc.tile_pool(name="qkr", bufs=2))
        v_pool = ctx.enter_context(tc.tile_pool(name="v", bufs=2))
        m_pool = ctx.enter_context(tc.tile_pool(name="m", bufs=1))
        e_pool = ctx.enter_context(tc.tile_pool(name="e", bufs=E_BUFS))
        o_pool = ctx.enter_context(tc.tile_pool(name="o", bufs=4))
        r_pool = ctx.enter_context(tc.tile_pool(name="r", bufs=4))
        sc_ps = ctx.enter_context(tc.tile_pool(name="sc", bufs=SC_BUFS, space="PSUM"))
        cx_ps = ctx.enter_context(tc.tile_pool(name="cx", bufs=CX_BUFS, space="PSUM"))

        mask_sb = None
        if n_mask_tiles:
            mask_sb = m_pool.tile([P, n_mask_tiles * P], F32, tag="mask")
            nc.sync.dma_start(mask_sb[:], mt_d[:])

        for slot in range(SLOTS_PER_CORE):
            status, uid_of = schedules[slot]
            if QK_MODE == "fp32r":
                qT32 = qk_pool.tile([P, SQ], F32, tag="q32")
                nc.sync.dma_start(qT32[:], qT_d[slot])
                kT32 = qk_pool.tile([P, SK], F32, tag="k32")
                nc.sync.dma_start(kT32[:], kT_d[slot])
                qT = qkr_pool.tile([P, SQ], F32R, tag="qr")
                nc.vector.tensor_copy(qT[:], qT32[:])
                kT = qkr_pool.tile([P, SK], F32R, tag="kr")
                nc.vector.tensor_copy(kT[:], kT32[:])
            elif QK_MODE == "fp16":
                # host supplies fp32; cast via DVE to fp16
                qT32 = qk_pool.tile([P, SQ], F32, tag="q32")
                nc.sync.dma_start(qT32[:], qT_d[slot])
                kT32 = qk_pool.tile([P, SK], F32, tag="k32")
                nc.sync.dma_start(kT32[:], kT_d[slot])
                qT = qkr_pool.tile([P, SQ], F16, tag="qr")
                nc.vector.tensor_copy(qT[:], qT32[:])
                kT = qkr_pool.tile([P, SK], F16, tag="kr")
                nc.vector.tensor_copy(kT[:], kT32[:])
            else:  # bf16x3
                qT32 = qk_pool.tile([P, SQ], F32, tag="q32")
                nc.sync.dma_start(qT32[:], qT_d[slot])
                kT32 = qk_pool.tile([P, SK], F32, tag="k32")
                nc.sync.dma_start(kT32[:], kT_d[slot])
                qhi = qkr_pool.tile([P, SQ], BF16, tag="qhi")
                nc.vector.tensor_copy(qhi[:], qT32[:])
                khi = qkr_pool.tile([P, SK], BF16, tag="khi")
                nc.vector.tensor_copy(khi[:], kT32[:])
                qhi32 = qkr_pool.tile([P, SQ], F32, tag="qhi32")
                nc.vector.tensor_copy(qhi32[:], qhi[:])
                khi32 = qkr_pool.tile([P, SK], F32, tag="khi32")
                nc.vector.tensor_copy(khi32[:], khi[:])
                qlo = qkr_pool.tile([P, SQ], BF16, tag="qlo")
                nc.vector.tensor_sub(qlo[:], qT32[:], qhi32[:])
                klo = qkr_pool.tile([P, SK], BF16, tag="klo")
                nc.vector.tensor_sub(klo[:], kT32[:], khi32[:])

            v_sb = v_pool.tile([P, N_SK_TILES * (HN + 1)], F16, tag="v")
            nc.sync.dma_start(
                v_sb[:], v_d[slot].rearrange("p t c -> p (t c)")
            )
            for ci in range(N_CHUNKS):
                i_tiles = [
                    i
                    for i in range(ci * CHUNK // P, (ci + 1) * CHUNK // P)
                    if any(status[j, i] != SKIP for j in range(N_SK_TILES))
                ]
                if not i_tiles:
                    continue
                # sk tiles needed for this sq chunk
                js = [
                    j
                    for j in range(N_SK_TILES)
                    if any(status[j, i] != SKIP for i in i_tiles)
                ]
                c0 = ci * CHUNK

                # group j's into PSUM group tiles of up to GROUP blocks
                exp_tiles: dict[int, tuple] = {}  # j -> (expT tile, col offset)
                for g0 in range(0, len(js), GROUP):
                    gjs = js[g0 : g0 + GROUP]
                    width = len(gjs) * CHUNK
                    sc = sc_ps.tile([P, GROUP * CHUNK], F32, tag="scores")
                    for k, j in enumerate(gjs):
                        co = k * CHUNK
                        if QK_MODE == "bf16x3":
                            nc.tensor.matmul(
                                sc[:, co : co + CHUNK],
                                khi[:, j * P : (j + 1) * P],
                                qhi[:, c0 : c0 + CHUNK],
                                start=True, stop=False,
                            )
                            nc.tensor.matmul(
                                sc[:, co : co + CHUNK],
                                khi[:, j * P : (j + 1) * P],
                                qlo[:, c0 : c0 + CHUNK],
                                start=False, stop=False,
                            )
                            nc.tensor.matmul(
                                sc[:, co : co + CHUNK],
                                klo[:, j * P : (j + 1) * P],
                                qhi[:, c0 : c0 + CHUNK],
                                start=False, stop=True,
                            )
                        else:
                            nc.tensor.matmul(
                                sc[:, co : co + CHUNK],
                                kT[:, j * P : (j + 1) * P],
                                qT[:, c0 : c0 + CHUNK],
                                start=True, stop=True,
                            )
                        # additive mask tiles for partial sub-blocks
                        for h, i in enumerate(range(ci * CHUNK // P, (ci + 1) * CHUNK // P)):
                            if status[j, i] == PARTIAL:
                                uid = uid_of[(j, i)]
                                nc.vector.tensor_add(
                                    sc[:, co + h * P : co + (h + 1) * P],
                                    sc[:, co + h * P : co + (h + 1) * P],
                                    mask_sb[:, uid * P : (uid + 1) * P],
                                )
                    et = e_pool.tile([P, GROUP * CHUNK], F16, tag="expT")
                    nc.scalar.activation(
                        et[:, :width], sc[:, :width],
                        mybir.ActivationFunctionType.Exp,
                        scale=inv_norm,
                    )
                    for k, j in enumerate(gjs):
                        exp_tiles[j] = (et, k * CHUNK)

                # PV per 128-wide sq tile of this chunk
                for ii, i in enumerate(i_tiles):
                    pv_js = [j for j in range(N_SK_TILES) if status[j, i] != SKIP]
                    cx = cx_ps.tile([P, HN + 1], F32, tag="ctx")
                    for idx, j in enumerate(pv_js):
                        et, co = exp_tiles[j]
                        icol = co + (i - ci * CHUNK // P) * P
                        nc.tensor.matmul(
                            cx[:],
                            et[:, icol : icol + P],
                            v_sb[:, j * (HN + 1) : (j + 1) * (HN + 1)],
                            start=(idx == 0),
                            stop=(idx == len(pv_js) - 1),
                        )
                    recip = r_pool.tile([P, 1], F32, tag="recip")
                    nc.vector.reciprocal(recip[:], cx[:, HN : HN + 1])
                    o_sb = o_pool.tile([P, HN], F32, tag="out")
                    nc.vector.tensor_scalar_mul(o_sb[:], cx[:, 0:HN], recip[:])
                    nc.sync.dma_start(out_d[slot, i], o_sb[:])

    nc.compile()
    return nc


_cache = {}


def _get_program(mask: np.ndarray):
    key = mask.tobytes()
    if key in _cache:
        return _cache[key]

    # schedules per batch; slots [0,1] -> b=0, [2,3] -> b=1 (same for all cores)
    scheds = []
    all_tiles: list[np.ndarray] = []
    tile_index: dict[bytes, int] = {}
    for b in range(B):
        status, uniq, uid_of = _block_schedule(np.asarray(mask[b, 0]))
        remap = {}
        for local_uid, t in enumerate(uniq):
            k = t.tobytes()
            if k not in tile_index:
                tile_index[k] = len(all_tiles)
                all_tiles.append(t)
            remap[local_uid] = tile_index[k]
        uid_of = {ji: remap[u] for ji, u in uid_of.items()}
        scheds.append((status, uid_of))

    slot_scheds = [scheds[0], scheds[0], scheds[1], scheds[1]]
    n_tiles = len(all_tiles)
    nc = _build_program(slot_scheds, n_tiles)

    if n_tiles:
        mt = np.stack(all_tiles)  # [U, 128, 128]
        mask_tiles = np.ascontiguousarray(mt.transpose(1, 0, 2)).reshape(
            P, n_tiles * P
        )
    else:
        mask_tiles = None
    _cache[key] = (nc, mask_tiles)
    return _cache[key]


def _core_slots(c):
    return [(0, 2 * c), (0, 2 * c + 1), (1, 2 * c), (1, 2 * c + 1)]


def prepare(query_layer, key_layer, value_layer, attention_mask):
    """Build (nc, in_maps). Shared by kernel() and the benchmark harness."""
    q = np.asarray(query_layer, dtype=np.float32)
    k = np.asarray(key_layer, dtype=np.float32)
    v = np.asarray(value_layer, dtype=np.float32)
    mask = np.asarray(attention_mask)

    nc, mask_tiles = _get_program(mask)

    # host layout prep
    # qT_all[b, n] = q[:, b, n, :].T  -> [B, NP, 128, SQ]
    qT_all = np.ascontiguousarray(q.transpose(1, 2, 3, 0))
    kT_all = np.ascontiguousarray(k.transpose(1, 2, 3, 0))
    # v_aug_all[b, n, p, t, c] = v[t*128+p, b, n, c], plus ones column
    v5 = v.reshape(N_SK_TILES, P, B, NP, HN).transpose(2, 3, 1, 0, 4)
    v_aug_all = np.empty((B, NP, P, N_SK_TILES, HN + 1), dtype=np.float16)
    v_aug_all[..., :HN] = v5
    v_aug_all[..., HN] = 1.0

    in_maps = []
    for c in range(N_CORES):
        slots = _core_slots(c)
        im = {
            "qT": np.ascontiguousarray(np.stack([qT_all[b, n] for b, n in slots])),
            "kT": np.ascontiguousarray(np.stack([kT_all[b, n] for b, n in slots])),
            "v_aug": np.ascontiguousarray(
                np.stack([v_aug_all[b, n] for b, n in slots])
            ),
        }
        if mask_tiles is not None:
            im["mask_tiles"] = mask_tiles
        in_maps.append(im)
    return nc, in_maps


def assemble(results):
    """Gather per-core 'out' arrays into the full [SQ, B, NP*HN] output."""
    full = np.empty((SQ, B, NP * HN), dtype=np.float32)
    for c in range(N_CORES):
        o = results[c]["out"]  # [4, 16, 128, 128]
        for s, (b, n) in enumerate(_core_slots(c)):
            full[:, b, n * HN : (n + 1) * HN] = o[s].reshape(SQ, HN)
    return full


def kernel(query_layer, key_layer, value_layer, attention_mask):
    from concourse.bass_utils import run_bass_kernel_spmd

    nc, in_maps = prepare(query_layer, key_layer, value_layer, attention_mask)
    res = run_bass_kernel_spmd(nc, in_maps, list(range(N_CORES)))
    return assemble(res.results)



# revision 7
# speedup vs baseline: 32021.3057x; 32021.3057x over previous
"""Causal multi-head attention forward on 8 Trainium2 NeuronCores.

Problem: nn_CoreAttention (SQ=SK=2048, B=2, NP=16 heads, HN=128, fp32).

Sharding: the 32 (batch, head) pairs are split 4 per core (tensor-parallel
over heads, data-parallel over batch). No collectives needed.

Per (b, n) pair the kernel computes, in transposed score orientation:
    scoresT[sk, sq] = (K Q^T)                 (PE matmul fp16, hn contracted)
    expT = exp(scoresT/sqrt(HN) + add_mask)   (ScalarE, fused scale, fp16 out)
    ctx_aug[sq, hn+1] = expT^T @ [V | 1]      (PE matmul, sk contracted;
                                               col hn holds the softmax denom)
    ctx = ctx_aug[:, :hn] * 1/ctx_aug[:, hn]  (DVE reciprocal + scale)

Q/K/V stream in as fp16 (host casts), context returns as fp16 and is
upcast on the host; the softmax accumulations stay fp32 in PSUM.

The block schedule (which 128x128 score blocks are skipped / masked) is
derived from the actual attention_mask at build time, so any mask pattern
produces a correct (if differently-sized) kernel. The causal mask gives the
standard lower-triangular schedule with one unique triangular additive tile.
"""

import math
import os
import numpy as np
from contextlib import ExitStack

import concourse.bacc as bacc
import concourse.tile as tile
from concourse import mybir

SQ, SK, B, NP, HN = 2048, 2048, 2, 16, 128
N_CORES = 8
SLOTS_PER_CORE = 4  # (b, n) pairs per core
P = 128             # partition dim / block size
CHUNK = int(os.environ.get("ATT_CHUNK", "256"))
GROUP = int(os.environ.get("ATT_GROUP", "4"))
SC_BUFS = int(os.environ.get("ATT_SC_BUFS", "3"))
CX_BUFS = int(os.environ.get("ATT_CX_BUFS", "2"))
E_BUFS = int(os.environ.get("ATT_E_BUFS", "6"))
POSTMASK = int(os.environ.get("ATT_POSTMASK", "0"))  # 0/1 multiply after exp
SWPIPE = int(os.environ.get("ATT_SWPIPE", "0"))      # emit PV one chunk behind QK
N_ITERS = int(os.environ.get("ATT_N_ITERS", "1"))  # in-NEFF timing loop count
N_SQ_TILES = SQ // P        # 16
N_SK_TILES = SK // P        # 16
N_CHUNKS = SQ // CHUNK
NEG = -60000.0              # additive mask value; exp -> exactly 0

F32 = mybir.dt.float32
F16 = mybir.dt.float16

SKIP, FULL, PARTIAL = 0, 1, 2


def _block_schedule(mask_b: np.ndarray):
    """Classify each 128x128 (sk_tile j, sq_tile i) block of one batch's mask.

    Returns (status[j][i], tiles) where tiles maps uid -> additive fp32
    [128(sk), 128(sq)] tile (transposed into scoresT orientation).
    """
    m4 = mask_b.reshape(N_SQ_TILES, P, N_SK_TILES, P)
    alls = m4.all(axis=(1, 3))  # [i, j]
    anys = m4.any(axis=(1, 3))
    status = np.zeros((N_SK_TILES, N_SQ_TILES), dtype=np.int64)
    tiles: dict[bytes, int] = {}
    uniq: list[np.ndarray] = []
    uid_of: dict[tuple[int, int], int] = {}
    for j in range(N_SK_TILES):
        for i in range(N_SQ_TILES):
            if alls[i, j]:
                status[j, i] = SKIP
            elif not anys[i, j]:
                status[j, i] = FULL
            else:
                status[j, i] = PARTIAL
                t = np.where(m4[i, :, j, :].T, np.float32(NEG), np.float32(0.0))
                key = t.tobytes()
                if key not in tiles:
                    tiles[key] = len(uniq)
                    uniq.append(t)
                uid_of[(j, i)] = tiles[key]
    return status, uniq, uid_of


def _build_body(nc, tc, ctx, schedules, mask_sb, qT_d, kT_d, v_d, out_d, pools):
    """Emit one full forward pass (all slots) into the program."""
    inv_norm = 1.0 / math.sqrt(HN)
    qk_pool, v_pool, e_pool, o_pool, r_pool, sc_ps, cx_ps = pools

    for slot in range(SLOTS_PER_CORE):
        status, uid_of = schedules[slot]
        qT = qk_pool.tile([P, SQ], F16, tag="q")
        nc.sync.dma_start(qT[:], qT_d[slot])
        kT = qk_pool.tile([P, SK], F16, tag="k")
        nc.sync.dma_start(kT[:], kT_d[slot])

        v_sb = v_pool.tile([P, N_SK_TILES * (HN + 1)], F16, tag="v")
        nc.sync.dma_start(v_sb[:], v_d[slot].rearrange("p t c -> p (t c)"))

        def emit_pv(ci, i_tiles, exp_tiles):
            # PV per 128-wide sq tile of this chunk
            for i in i_tiles:
                pv_js = [j for j in range(N_SK_TILES) if status[j, i] != SKIP]
                cx = cx_ps.tile([P, HN + 1], F32, tag="ctx")
                for idx, j in enumerate(pv_js):
                    et, co = exp_tiles[j]
                    icol = co + (i - ci * CHUNK // P) * P
                    nc.tensor.matmul(
                        cx[:],
                        et[:, icol : icol + P],
                        v_sb[:, j * (HN + 1) : (j + 1) * (HN + 1)],
                        start=(idx == 0),
                        stop=(idx == len(pv_js) - 1),
                    )
                recip = r_pool.tile([P, 1], F32, tag="recip")
                nc.vector.reciprocal(recip[:], cx[:, HN : HN + 1])
                o_sb = o_pool.tile([P, HN], F16, tag="out")
                nc.any.tensor_scalar_mul(o_sb[:], cx[:, 0:HN], recip[:])
                nc.sync.dma_start(out_d[slot, i], o_sb[:])

        pending_pv = None
        for ci in range(N_CHUNKS):
            i_tiles = [
                i
                for i in range(ci * CHUNK // P, (ci + 1) * CHUNK // P)
                if any(status[j, i] != SKIP for j in range(N_SK_TILES))
            ]
            if not i_tiles:
                continue
            js = [
                j
                for j in range(N_SK_TILES)
                if any(status[j, i] != SKIP for i in i_tiles)
            ]
            c0 = ci * CHUNK

            # group j's into PSUM group tiles of up to GROUP blocks
            exp_tiles: dict[int, tuple] = {}  # j -> (expT tile, col offset)
            for g0 in range(0, len(js), GROUP):
                gjs = js[g0 : g0 + GROUP]
                width = len(gjs) * CHUNK
                sc = sc_ps.tile([P, GROUP * CHUNK], F32, tag="scores")
                for k, j in enumerate(gjs):
                    co = k * CHUNK
                    nc.tensor.matmul(
                        sc[:, co : co + CHUNK],
                        kT[:, j * P : (j + 1) * P],
                        qT[:, c0 : c0 + CHUNK],
                        start=True, stop=True,
                    )
                    if not POSTMASK:
                        # additive mask tiles for partial sub-blocks
                        for h, i in enumerate(
                            range(ci * CHUNK // P, (ci + 1) * CHUNK // P)
                        ):
                            if status[j, i] == PARTIAL:
                                uid = uid_of[(j, i)]
                                nc.any.tensor_add(
                                    sc[:, co + h * P : co + (h + 1) * P],
                                    sc[:, co + h * P : co + (h + 1) * P],
                                    mask_sb[:, uid * P : (uid + 1) * P],
                                )
                et = e_pool.tile([P, GROUP * CHUNK], F16, tag="expT")
                nc.scalar.activation(
                    et[:, :width], sc[:, :width],
                    mybir.ActivationFunctionType.Exp,
                    scale=inv_norm,
                )
                if POSTMASK:
                    # zero masked entries after exp (0/1 fp16 mask tiles)
                    for k, j in enumerate(gjs):
                        co = k * CHUNK
                        for h, i in enumerate(
                            range(ci * CHUNK // P, (ci + 1) * CHUNK // P)
                        ):
                            if status[j, i] == PARTIAL:
                                uid = uid_of[(j, i)]
                                nc.any.tensor_mul(
                                    et[:, co + h * P : co + (h + 1) * P],
                                    et[:, co + h * P : co + (h + 1) * P],
                                    mask_sb[:, uid * P : (uid + 1) * P],
                                )
                for k, j in enumerate(gjs):
                    exp_tiles[j] = (et, k * CHUNK)

            if SWPIPE:
                if pending_pv is not None:
                    emit_pv(*pending_pv)
                pending_pv = (ci, i_tiles, exp_tiles)
            else:
                emit_pv(ci, i_tiles, exp_tiles)
        if pending_pv is not None:
            emit_pv(*pending_pv)


def _build_program(schedules, n_mask_tiles, n_iters=1):
    """Build the SPMD bass program. schedules[slot] = (status, uid_of).

    n_iters > 1 wraps the whole forward pass in a hardware loop so one NEFF
    execution runs the kernel n_iters times back-to-back (for HW timing).
    """
    nc = bacc.Bacc()

    qT_d = nc.declare_dram_parameter("qT", [SLOTS_PER_CORE, P, SQ], F16, isOutput=False)
    kT_d = nc.declare_dram_parameter("kT", [SLOTS_PER_CORE, P, SK], F16, isOutput=False)
    v_d = nc.declare_dram_parameter(
        "v_aug", [SLOTS_PER_CORE, P, N_SK_TILES, HN + 1], F16, isOutput=False
    )
    mt_d = None
    if n_mask_tiles:
        mt_d = nc.declare_dram_parameter(
            "mask_tiles",
            [P, n_mask_tiles * P],
            F16 if POSTMASK else F32,
            isOutput=False,
        )
    out_d = nc.declare_dram_parameter(
        "out", [SLOTS_PER_CORE, N_SQ_TILES, P, HN], F16, isOutput=True
    )

    with tile.TileContext(nc) as tc, ExitStack() as ctx:
        qk_pool = ctx.enter_context(tc.tile_pool(name="qk", bufs=2))
        v_pool = ctx.enter_context(tc.tile_pool(name="v", bufs=2))
        m_pool = ctx.enter_context(tc.tile_pool(name="m", bufs=1))
        e_pool = ctx.enter_context(tc.tile_pool(name="e", bufs=E_BUFS))
        o_pool = ctx.enter_context(tc.tile_pool(name="o", bufs=4))
        r_pool = ctx.enter_context(tc.tile_pool(name="r", bufs=4))
        sc_ps = ctx.enter_context(tc.tile_pool(name="sc", bufs=SC_BUFS, space="PSUM"))
        cx_ps = ctx.enter_context(tc.tile_pool(name="cx", bufs=CX_BUFS, space="PSUM"))

        mask_sb = None
        if n_mask_tiles:
            mask_sb = m_pool.tile(
                [P, n_mask_tiles * P], F16 if POSTMASK else F32, tag="mask"
            )
            nc.sync.dma_start(mask_sb[:], mt_d[:])

        pools = (qk_pool, v_pool, e_pool, o_pool, r_pool, sc_ps, cx_ps)
        if n_iters == 1:
            _build_body(nc, tc, ctx, schedules, mask_sb, qT_d, kT_d, v_d, out_d, pools)
        else:
            with tc.For_i(0, n_iters):
                _build_body(
                    nc, tc, ctx, schedules, mask_sb, qT_d, kT_d, v_d, out_d, pools
                )

    nc.compile()
    return nc


_cache = {}


def _get_program(mask: np.ndarray, n_iters=1):
    key = (mask.tobytes(), n_iters)
    if key in _cache:
        return _cache[key]

    # schedules per batch; slots [0,1] -> b=0, [2,3] -> b=1 (same for all cores)
    scheds = []
    all_tiles: list[np.ndarray] = []
    tile_index: dict[bytes, int] = {}
    for b in range(B):
        status, uniq, uid_of = _block_schedule(np.asarray(mask[b, 0]))
        remap = {}
        for local_uid, t in enumerate(uniq):
            k = t.tobytes()
            if k not in tile_index:
                tile_index[k] = len(all_tiles)
                all_tiles.append(t)
            remap[local_uid] = tile_index[k]
        uid_of = {ji: remap[u] for ji, u in uid_of.items()}
        scheds.append((status, uid_of))

    slot_scheds = [scheds[0], scheds[0], scheds[1], scheds[1]]
    n_tiles = len(all_tiles)
    nc = _build_program(slot_scheds, n_tiles, n_iters=n_iters)

    if n_tiles:
        mt = np.stack(all_tiles)  # [U, 128, 128] additive (0 / NEG)
        if POSTMASK:
            mt = (mt == 0.0).astype(np.float16)  # multiplicative 1/0
        mask_tiles = np.ascontiguousarray(mt.transpose(1, 0, 2)).reshape(
            P, n_tiles * P
        )
    else:
        mask_tiles = None
    _cache[key] = (nc, mask_tiles)
    return _cache[key]


def _core_slots(c):
    return [(0, 2 * c), (0, 2 * c + 1), (1, 2 * c), (1, 2 * c + 1)]


def prepare(query_layer, key_layer, value_layer, attention_mask, n_iters=None):
    """Build (nc, in_maps). Shared by kernel() and the benchmark harness."""
    if n_iters is None:
        n_iters = N_ITERS
    q = np.asarray(query_layer, dtype=np.float32)
    k = np.asarray(key_layer, dtype=np.float32)
    v = np.asarray(value_layer, dtype=np.float32)
    mask = np.asarray(attention_mask)

    nc, mask_tiles = _get_program(mask, n_iters)

    # host layout prep (cast to fp16 for half the HBM traffic; accumulations
    # on device stay fp32 in PSUM)
    # qT_all[b, n] = q[:, b, n, :].T  -> [B, NP, 128, SQ]
    qT_all = np.ascontiguousarray(q.transpose(1, 2, 3, 0), dtype=np.float16)
    kT_all = np.ascontiguousarray(k.transpose(1, 2, 3, 0), dtype=np.float16)
    # v_aug_all[b, n, p, t, c] = v[t*128+p, b, n, c], plus ones column
    v5 = v.reshape(N_SK_TILES, P, B, NP, HN).transpose(2, 3, 1, 0, 4)
    v_aug_all = np.empty((B, NP, P, N_SK_TILES, HN + 1), dtype=np.float16)
    v_aug_all[..., :HN] = v5
    v_aug_all[..., HN] = 1.0

    in_maps = []
    for c in range(N_CORES):
        slots = _core_slots(c)
        im = {
            "qT": np.ascontiguousarray(np.stack([qT_all[b, n] for b, n in slots])),
            "kT": np.ascontiguousarray(np.stack([kT_all[b, n] for b, n in slots])),
            "v_aug": np.ascontiguousarray(
                np.stack([v_aug_all[b, n] for b, n in slots])
            ),
        }
        if mask_tiles is not None:
            im["mask_tiles"] = mask_tiles
        in_maps.append(im)
    return nc, in_maps


def assemble(results):
    """Gather per-core 'out' arrays into the full [SQ, B, NP*HN] output."""
    full = np.empty((SQ, B, NP * HN), dtype=np.float32)
    for c in range(N_CORES):
        o = results[c]["out"]  # [4, 16, 128, 128] fp16
        for s, (b, n) in enumerate(_core_slots(c)):
            full[:, b, n * HN : (n + 1) * HN] = o[s].reshape(SQ, HN).astype(
                np.float32
            )
    return full


def kernel(query_layer, key_layer, value_layer, attention_mask):
    from concourse.bass_utils import run_bass_kernel_spmd

    nc, in_maps = prepare(
        query_layer, key_layer, value_layer, attention_mask, n_iters=1
    )
    res = run_bass_kernel_spmd(nc, in_maps, list(range(N_CORES)))
    return assemble(res.results)


# revision 13
# speedup vs baseline: 35880.9884x; 1.1205x over previous
"""Causal multi-head attention forward on 8 Trainium2 NeuronCores.

Problem: nn_CoreAttention (SQ=SK=2048, B=2, NP=16 heads, HN=128, fp32).

Sharding: the 32 (batch, head) pairs are split 4 per core (tensor-parallel
over heads, data-parallel over batch). No collectives needed.

Per (b, n) pair the kernel computes, in transposed score orientation:
    scoresT[sk, sq] = (K Q^T)                 (PE matmul fp16, hn contracted)
    expT = exp(scoresT/sqrt(HN) + add_mask)   (ScalarE, fused scale, fp16 out)
    ctx_aug[sq, hn+1] = expT^T @ [V | 1]      (PE matmul, sk contracted;
                                               col hn holds the softmax denom)
    ctx = ctx_aug[:, :hn] * 1/ctx_aug[:, hn]  (DVE reciprocal + scale)

Q/K/V stream in as fp16 (host casts), context returns as fp16 and is
upcast on the host; the softmax accumulations stay fp32 in PSUM.

The block schedule (which 128x128 score blocks are skipped / masked) is
derived from the actual attention_mask at build time, so any mask pattern
produces a correct (if differently-sized) kernel. The causal mask gives the
standard lower-triangular schedule with one unique triangular additive tile.
"""

import math
import os
import numpy as np
from contextlib import ExitStack

import concourse.bacc as bacc
import concourse.tile as tile
from concourse import mybir

SQ, SK, B, NP, HN = 2048, 2048, 2, 16, 128
N_CORES = 8
SLOTS_PER_CORE = 4  # (b, n) pairs per core
P = 128             # partition dim / block size
CHUNK = int(os.environ.get("ATT_CHUNK", "256"))
GROUP = int(os.environ.get("ATT_GROUP", "4"))
SC_BUFS = int(os.environ.get("ATT_SC_BUFS", "3"))
CX_BUFS = int(os.environ.get("ATT_CX_BUFS", "2"))
E_BUFS = int(os.environ.get("ATT_E_BUFS", "8"))
POSTMASK = int(os.environ.get("ATT_POSTMASK", "1"))  # 0/1 multiply after exp
SWPIPE = int(os.environ.get("ATT_SWPIPE", "1"))      # emit PV one chunk behind QK
QK_BUFS = int(os.environ.get("ATT_QK_BUFS", "4"))    # q/k tiles (2 per slot)
INTERLEAVE = int(os.environ.get("ATT_INTERLEAVE", "1"))  # slots emitted per round
N_ITERS = int(os.environ.get("ATT_N_ITERS", "1"))  # in-NEFF timing loop count
N_SQ_TILES = SQ // P        # 16
N_SK_TILES = SK // P        # 16
N_CHUNKS = SQ // CHUNK
NEG = -60000.0              # additive mask value; exp -> exactly 0

F32 = mybir.dt.float32
F16 = mybir.dt.float16

SKIP, FULL, PARTIAL = 0, 1, 2


def _block_schedule(mask_b: np.ndarray):
    """Classify each 128x128 (sk_tile j, sq_tile i) block of one batch's mask.

    Returns (status[j][i], tiles) where tiles maps uid -> additive fp32
    [128(sk), 128(sq)] tile (transposed into scoresT orientation).
    """
    m4 = mask_b.reshape(N_SQ_TILES, P, N_SK_TILES, P)
    alls = m4.all(axis=(1, 3))  # [i, j]
    anys = m4.any(axis=(1, 3))
    status = np.zeros((N_SK_TILES, N_SQ_TILES), dtype=np.int64)
    tiles: dict[bytes, int] = {}
    uniq: list[np.ndarray] = []
    uid_of: dict[tuple[int, int], int] = {}
    for j in range(N_SK_TILES):
        for i in range(N_SQ_TILES):
            if alls[i, j]:
                status[j, i] = SKIP
            elif not anys[i, j]:
                status[j, i] = FULL
            else:
                status[j, i] = PARTIAL
                t = np.where(m4[i, :, j, :].T, np.float32(NEG), np.float32(0.0))
                key = t.tobytes()
                if key not in tiles:
                    tiles[key] = len(uniq)
                    uniq.append(t)
                uid_of[(j, i)] = tiles[key]
    return status, uniq, uid_of


def _build_body(nc, tc, ctx, schedules, mask_sb, qT_d, kT_d, v_d, out_d, pools):
    """Emit one full forward pass (all slots) into the program.

    Each slot is a generator yielding once per chunk; INTERLEAVE slots are
    driven round-robin so the scheduler sees independent work to fill
    pipeline stalls.
    """
    inv_norm = 1.0 / math.sqrt(HN)
    qk_pool, v_pool, e_pool, o_pool, r_pool, sc_ps, cx_ps = pools

    def slot_gen(slot):
        status, uid_of = schedules[slot]
        qT = qk_pool.tile([P, SQ], F16, tag="q")
        nc.sync.dma_start(qT[:], qT_d[slot])
        kT = qk_pool.tile([P, SK], F16, tag="k")
        nc.sync.dma_start(kT[:], kT_d[slot])

        v_sb = v_pool.tile([P, N_SK_TILES * (HN + 1)], F16, tag="v")
        nc.sync.dma_start(v_sb[:], v_d[slot].rearrange("p t c -> p (t c)"))

        def emit_pv(ci, i_tiles, exp_tiles):
            # PV per 128-wide sq tile of this chunk
            for i in i_tiles:
                pv_js = [j for j in range(N_SK_TILES) if status[j, i] != SKIP]
                cx = cx_ps.tile([P, HN + 1], F32, tag="ctx")
                for idx, j in enumerate(pv_js):
                    et, co = exp_tiles[j]
                    icol = co + (i - ci * CHUNK // P) * P
                    nc.tensor.matmul(
                        cx[:],
                        et[:, icol : icol + P],
                        v_sb[:, j * (HN + 1) : (j + 1) * (HN + 1)],
                        start=(idx == 0),
                        stop=(idx == len(pv_js) - 1),
                    )
                recip = r_pool.tile([P, 1], F32, tag="recip")
                nc.vector.reciprocal(recip[:], cx[:, HN : HN + 1])
                o_sb = o_pool.tile([P, HN], F16, tag="out")
                nc.any.tensor_scalar_mul(o_sb[:], cx[:, 0:HN], recip[:])
                nc.sync.dma_start(out_d[slot, i], o_sb[:])

        pending_pv = None
        for ci in range(N_CHUNKS):
            yield
            i_tiles = [
                i
                for i in range(ci * CHUNK // P, (ci + 1) * CHUNK // P)
                if any(status[j, i] != SKIP for j in range(N_SK_TILES))
            ]
            if not i_tiles:
                continue
            js = [
                j
                for j in range(N_SK_TILES)
                if any(status[j, i] != SKIP for i in i_tiles)
            ]
            c0 = ci * CHUNK

            # group j's into PSUM group tiles of up to GROUP blocks
            exp_tiles: dict[int, tuple] = {}  # j -> (expT tile, col offset)
            for g0 in range(0, len(js), GROUP):
                gjs = js[g0 : g0 + GROUP]
                width = len(gjs) * CHUNK
                sc = sc_ps.tile([P, GROUP * CHUNK], F32, tag="scores")
                for k, j in enumerate(gjs):
                    co = k * CHUNK
                    nc.tensor.matmul(
                        sc[:, co : co + CHUNK],
                        kT[:, j * P : (j + 1) * P],
                        qT[:, c0 : c0 + CHUNK],
                        start=True, stop=True,
                    )
                    if not POSTMASK:
                        # additive mask tiles for partial sub-blocks
                        for h, i in enumerate(
                            range(ci * CHUNK // P, (ci + 1) * CHUNK // P)
                        ):
                            if status[j, i] == PARTIAL:
                                uid = uid_of[(j, i)]
                                nc.any.tensor_add(
                                    sc[:, co + h * P : co + (h + 1) * P],
                                    sc[:, co + h * P : co + (h + 1) * P],
                                    mask_sb[:, uid * P : (uid + 1) * P],
                                )
                et = e_pool.tile([P, GROUP * CHUNK], F16, tag="expT")
                nc.scalar.activation(
                    et[:, :width], sc[:, :width],
                    mybir.ActivationFunctionType.Exp,
                    scale=inv_norm,
                )
                if POSTMASK:
                    # zero masked entries after exp (0/1 fp16 mask tiles)
                    for k, j in enumerate(gjs):
                        co = k * CHUNK
                        for h, i in enumerate(
                            range(ci * CHUNK // P, (ci + 1) * CHUNK // P)
                        ):
                            if status[j, i] == PARTIAL:
                                uid = uid_of[(j, i)]
                                nc.any.tensor_mul(
                                    et[:, co + h * P : co + (h + 1) * P],
                                    et[:, co + h * P : co + (h + 1) * P],
                                    mask_sb[:, uid * P : (uid + 1) * P],
                                )
                for k, j in enumerate(gjs):
                    exp_tiles[j] = (et, k * CHUNK)

            if SWPIPE:
                if pending_pv is not None:
                    emit_pv(*pending_pv)
                pending_pv = (ci, i_tiles, exp_tiles)
            else:
                emit_pv(ci, i_tiles, exp_tiles)
        if pending_pv is not None:
            emit_pv(*pending_pv)

    for s0 in range(0, SLOTS_PER_CORE, INTERLEAVE):
        gens = [slot_gen(s) for s in range(s0, min(s0 + INTERLEAVE, SLOTS_PER_CORE))]
        while gens:
            gens = [g for g in gens if next(g, StopIteration) is not StopIteration]


def _build_program(schedules, n_mask_tiles, n_iters=1):
    """Build the SPMD bass program. schedules[slot] = (status, uid_of).

    n_iters > 1 wraps the whole forward pass in a hardware loop so one NEFF
    execution runs the kernel n_iters times back-to-back (for HW timing).
    """
    nc = bacc.Bacc()

    qT_d = nc.declare_dram_parameter("qT", [SLOTS_PER_CORE, P, SQ], F16, isOutput=False)
    kT_d = nc.declare_dram_parameter("kT", [SLOTS_PER_CORE, P, SK], F16, isOutput=False)
    v_d = nc.declare_dram_parameter(
        "v_aug", [SLOTS_PER_CORE, P, N_SK_TILES, HN + 1], F16, isOutput=False
    )
    mt_d = None
    if n_mask_tiles:
        mt_d = nc.declare_dram_parameter(
            "mask_tiles",
            [P, n_mask_tiles * P],
            F16 if POSTMASK else F32,
            isOutput=False,
        )
    out_d = nc.declare_dram_parameter(
        "out", [SLOTS_PER_CORE, N_SQ_TILES, P, HN], F16, isOutput=True
    )

    with tile.TileContext(nc) as tc, ExitStack() as ctx:
        qk_pool = ctx.enter_context(tc.tile_pool(name="qk", bufs=QK_BUFS))
        v_pool = ctx.enter_context(
            tc.tile_pool(name="v", bufs=max(2, INTERLEAVE + 1))
        )
        m_pool = ctx.enter_context(tc.tile_pool(name="m", bufs=1))
        e_pool = ctx.enter_context(tc.tile_pool(name="e", bufs=E_BUFS))
        o_pool = ctx.enter_context(tc.tile_pool(name="o", bufs=4))
        r_pool = ctx.enter_context(tc.tile_pool(name="r", bufs=4))
        sc_ps = ctx.enter_context(tc.tile_pool(name="sc", bufs=SC_BUFS, space="PSUM"))
        cx_ps = ctx.enter_context(tc.tile_pool(name="cx", bufs=CX_BUFS, space="PSUM"))

        mask_sb = None
        if n_mask_tiles:
            mask_sb = m_pool.tile(
                [P, n_mask_tiles * P], F16 if POSTMASK else F32, tag="mask"
            )
            nc.sync.dma_start(mask_sb[:], mt_d[:])

        pools = (qk_pool, v_pool, e_pool, o_pool, r_pool, sc_ps, cx_ps)
        if n_iters == 1:
            _build_body(nc, tc, ctx, schedules, mask_sb, qT_d, kT_d, v_d, out_d, pools)
        else:
            with tc.For_i(0, n_iters):
                _build_body(
                    nc, tc, ctx, schedules, mask_sb, qT_d, kT_d, v_d, out_d, pools
                )

    nc.compile()
    return nc


_cache = {}


def _get_program(mask: np.ndarray, n_iters=1):
    key = (mask.tobytes(), n_iters)
    if key in _cache:
        return _cache[key]

    # schedules per batch; slots [0,1] -> b=0, [2,3] -> b=1 (same for all cores)
    scheds = []
    all_tiles: list[np.ndarray] = []
    tile_index: dict[bytes, int] = {}
    for b in range(B):
        status, uniq, uid_of = _block_schedule(np.asarray(mask[b, 0]))
        remap = {}
        for local_uid, t in enumerate(uniq):
            k = t.tobytes()
            if k not in tile_index:
                tile_index[k] = len(all_tiles)
                all_tiles.append(t)
            remap[local_uid] = tile_index[k]
        uid_of = {ji: remap[u] for ji, u in uid_of.items()}
        scheds.append((status, uid_of))

    slot_scheds = [scheds[0], scheds[0], scheds[1], scheds[1]]
    n_tiles = len(all_tiles)
    nc = _build_program(slot_scheds, n_tiles, n_iters=n_iters)

    if n_tiles:
        mt = np.stack(all_tiles)  # [U, 128, 128] additive (0 / NEG)
        if POSTMASK:
            mt = (mt == 0.0).astype(np.float16)  # multiplicative 1/0
        mask_tiles = np.ascontiguousarray(mt.transpose(1, 0, 2)).reshape(
            P, n_tiles * P
        )
    else:
        mask_tiles = None
    _cache[key] = (nc, mask_tiles)
    return _cache[key]


def _core_slots(c):
    return [(0, 2 * c), (0, 2 * c + 1), (1, 2 * c), (1, 2 * c + 1)]


def prepare(query_layer, key_layer, value_layer, attention_mask, n_iters=None):
    """Build (nc, in_maps). Shared by kernel() and the benchmark harness."""
    if n_iters is None:
        n_iters = N_ITERS
    q = np.asarray(query_layer, dtype=np.float32)
    k = np.asarray(key_layer, dtype=np.float32)
    v = np.asarray(value_layer, dtype=np.float32)
    mask = np.asarray(attention_mask)

    nc, mask_tiles = _get_program(mask, n_iters)

    # host layout prep (cast to fp16 for half the HBM traffic; accumulations
    # on device stay fp32 in PSUM)
    # qT_all[b, n] = q[:, b, n, :].T  -> [B, NP, 128, SQ]
    qT_all = np.ascontiguousarray(q.transpose(1, 2, 3, 0), dtype=np.float16)
    kT_all = np.ascontiguousarray(k.transpose(1, 2, 3, 0), dtype=np.float16)
    # v_aug_all[b, n, p, t, c] = v[t*128+p, b, n, c], plus ones column
    v5 = v.reshape(N_SK_TILES, P, B, NP, HN).transpose(2, 3, 1, 0, 4)
    v_aug_all = np.empty((B, NP, P, N_SK_TILES, HN + 1), dtype=np.float16)
    v_aug_all[..., :HN] = v5
    v_aug_all[..., HN] = 1.0

    in_maps = []
    for c in range(N_CORES):
        slots = _core_slots(c)
        im = {
            "qT": np.ascontiguousarray(np.stack([qT_all[b, n] for b, n in slots])),
            "kT": np.ascontiguousarray(np.stack([kT_all[b, n] for b, n in slots])),
            "v_aug": np.ascontiguousarray(
                np.stack([v_aug_all[b, n] for b, n in slots])
            ),
        }
        if mask_tiles is not None:
            im["mask_tiles"] = mask_tiles
        in_maps.append(im)
    return nc, in_maps


def assemble(results):
    """Gather per-core 'out' arrays into the full [SQ, B, NP*HN] output."""
    full = np.empty((SQ, B, NP * HN), dtype=np.float32)
    for c in range(N_CORES):
        o = results[c]["out"]  # [4, 16, 128, 128] fp16
        for s, (b, n) in enumerate(_core_slots(c)):
            full[:, b, n * HN : (n + 1) * HN] = o[s].reshape(SQ, HN).astype(
                np.float32
            )
    return full


def kernel(query_layer, key_layer, value_layer, attention_mask):
    from concourse.bass_utils import run_bass_kernel_spmd

    nc, in_maps = prepare(
        query_layer, key_layer, value_layer, attention_mask, n_iters=1
    )
    res = run_bass_kernel_spmd(nc, in_maps, list(range(N_CORES)))
    return assemble(res.results)


# revision 20
# speedup vs baseline: 36120.7788x; 1.0067x over previous
"""Causal multi-head attention forward on 8 Trainium2 NeuronCores.

Problem: nn_CoreAttention (SQ=SK=2048, B=2, NP=16 heads, HN=128, fp32).

Sharding: the 32 (batch, head) pairs are split 4 per core (tensor-parallel
over heads, data-parallel over batch). No collectives needed.

Per (b, n) pair the kernel computes, in transposed score orientation:
    scoresT[sk, sq] = (K Q^T)                 (PE matmul fp16, hn contracted)
    expT = exp(scoresT/sqrt(HN) + add_mask)   (ScalarE, fused scale, fp16 out)
    ctx_aug[sq, hn+1] = expT^T @ [V | 1]      (PE matmul, sk contracted;
                                               col hn holds the softmax denom)
    ctx = ctx_aug[:, :hn] * 1/ctx_aug[:, hn]  (DVE reciprocal + scale)

Q/K/V stream in as fp16 (host casts), context returns as fp16 and is
upcast on the host; the softmax accumulations stay fp32 in PSUM.

The block schedule (which 128x128 score blocks are skipped / masked) is
derived from the actual attention_mask at build time, so any mask pattern
produces a correct (if differently-sized) kernel. The causal mask gives the
standard lower-triangular schedule with one unique triangular additive tile.
"""

import math
import os
import numpy as np
from contextlib import ExitStack

import concourse.bacc as bacc
import concourse.tile as tile
from concourse import mybir

SQ, SK, B, NP, HN = 2048, 2048, 2, 16, 128
N_CORES = 8
SLOTS_PER_CORE = 4  # (b, n) pairs per core
P = 128             # partition dim / block size
CHUNK = int(os.environ.get("ATT_CHUNK", "256"))
GROUP = int(os.environ.get("ATT_GROUP", "4"))
SC_BUFS = int(os.environ.get("ATT_SC_BUFS", "3"))
CX_BUFS = int(os.environ.get("ATT_CX_BUFS", "2"))
E_BUFS = int(os.environ.get("ATT_E_BUFS", "8"))
POSTMASK = int(os.environ.get("ATT_POSTMASK", "1"))  # 0/1 multiply after exp
SWPIPE = int(os.environ.get("ATT_SWPIPE", "1"))      # PV chunks emitted this far behind QK
QK_BUFS = int(os.environ.get("ATT_QK_BUFS", "4"))    # q/k tiles (2 per slot)
INTERLEAVE = int(os.environ.get("ATT_INTERLEAVE", "1"))  # slots emitted per round
TRIM = int(os.environ.get("ATT_TRIM", "0"))          # skip fully-masked sub-blocks
ALTREV = int(os.environ.get("ATT_ALTREV", "0"))      # reverse chunk order on odd slots
N_ITERS = int(os.environ.get("ATT_N_ITERS", "1"))  # in-NEFF timing loop count
N_SQ_TILES = SQ // P        # 16
N_SK_TILES = SK // P        # 16
N_CHUNKS = SQ // CHUNK
NEG = -60000.0              # additive mask value; exp -> exactly 0

F32 = mybir.dt.float32
F16 = mybir.dt.float16

SKIP, FULL, PARTIAL = 0, 1, 2


def _block_schedule(mask_b: np.ndarray):
    """Classify each 128x128 (sk_tile j, sq_tile i) block of one batch's mask.

    Returns (status[j][i], tiles) where tiles maps uid -> additive fp32
    [128(sk), 128(sq)] tile (transposed into scoresT orientation).
    """
    m4 = mask_b.reshape(N_SQ_TILES, P, N_SK_TILES, P)
    alls = m4.all(axis=(1, 3))  # [i, j]
    anys = m4.any(axis=(1, 3))
    status = np.zeros((N_SK_TILES, N_SQ_TILES), dtype=np.int64)
    tiles: dict[bytes, int] = {}
    uniq: list[np.ndarray] = []
    uid_of: dict[tuple[int, int], int] = {}
    for j in range(N_SK_TILES):
        for i in range(N_SQ_TILES):
            if alls[i, j]:
                status[j, i] = SKIP
            elif not anys[i, j]:
                status[j, i] = FULL
            else:
                status[j, i] = PARTIAL
                t = np.where(m4[i, :, j, :].T, np.float32(NEG), np.float32(0.0))
                key = t.tobytes()
                if key not in tiles:
                    tiles[key] = len(uniq)
                    uniq.append(t)
                uid_of[(j, i)] = tiles[key]
    return status, uniq, uid_of


def _build_body(nc, tc, ctx, schedules, mask_sb, qT_d, kT_d, v_d, out_d, pools):
    """Emit one full forward pass (all slots) into the program.

    Each slot is a generator yielding once per chunk; INTERLEAVE slots are
    driven round-robin so the scheduler sees independent work to fill
    pipeline stalls.
    """
    inv_norm = 1.0 / math.sqrt(HN)
    qk_pool, v_pool, e_pool, o_pool, r_pool, sc_ps, cx_ps = pools

    def slot_gen(slot):
        status, uid_of = schedules[slot]
        qT = qk_pool.tile([P, SQ], F16, tag="q")
        nc.sync.dma_start(qT[:], qT_d[slot])
        kT = qk_pool.tile([P, SK], F16, tag="k")
        nc.sync.dma_start(kT[:], kT_d[slot])

        v_sb = v_pool.tile([P, N_SK_TILES * (HN + 1)], F16, tag="v")
        nc.sync.dma_start(v_sb[:], v_d[slot].rearrange("p t c -> p (t c)"))

        def emit_pv(ci, i_tiles, exp_tiles):
            # PV per 128-wide sq tile of this chunk
            for i in i_tiles:
                pv_js = [j for j in range(N_SK_TILES) if status[j, i] != SKIP]
                cx = cx_ps.tile([P, HN + 1], F32, tag="ctx")
                for idx, j in enumerate(pv_js):
                    et, co = exp_tiles[j]
                    icol = co + (i - ci * CHUNK // P) * P
                    nc.tensor.matmul(
                        cx[:],
                        et[:, icol : icol + P],
                        v_sb[:, j * (HN + 1) : (j + 1) * (HN + 1)],
                        start=(idx == 0),
                        stop=(idx == len(pv_js) - 1),
                    )
                recip = r_pool.tile([P, 1], F32, tag="recip")
                nc.vector.reciprocal(recip[:], cx[:, HN : HN + 1])
                o_sb = o_pool.tile([P, HN], F16, tag="out")
                nc.any.tensor_scalar_mul(o_sb[:], cx[:, 0:HN], recip[:])
                nc.sync.dma_start(out_d[slot, i], o_sb[:])

        pending_pv = []
        chunk_order = (
            range(N_CHUNKS - 1, -1, -1)
            if (ALTREV and slot % 2 == 1)
            else range(N_CHUNKS)
        )
        for ci in chunk_order:
            yield
            i_tiles = [
                i
                for i in range(ci * CHUNK // P, (ci + 1) * CHUNK // P)
                if any(status[j, i] != SKIP for j in range(N_SK_TILES))
            ]
            if not i_tiles:
                continue
            js = [
                j
                for j in range(N_SK_TILES)
                if any(status[j, i] != SKIP for i in i_tiles)
            ]
            c0 = ci * CHUNK
            sub_is = list(range(ci * CHUNK // P, (ci + 1) * CHUNK // P))

            # group j's into PSUM group tiles of up to GROUP blocks
            exp_tiles: dict[int, tuple] = {}  # j -> (expT tile, col offset)
            for g0 in range(0, len(js), GROUP):
                gjs = js[g0 : g0 + GROUP]
                width = len(gjs) * CHUNK
                sc = sc_ps.tile([P, GROUP * CHUNK], F32, tag="scores")
                # live[j-block, sub-tile] cells of this group's sc tile
                live = [
                    [status[j, i] != SKIP for i in sub_is] for j in gjs
                ]
                for k, j in enumerate(gjs):
                    co = k * CHUNK
                    if TRIM:
                        # matmul only the live sub-range (128-aligned)
                        hs = [h for h, lv in enumerate(live[k]) if lv]
                        lo, hi = hs[0] * P, (hs[-1] + 1) * P
                    else:
                        lo, hi = 0, CHUNK
                    nc.tensor.matmul(
                        sc[:, co + lo : co + hi],
                        kT[:, j * P : (j + 1) * P],
                        qT[:, c0 + lo : c0 + hi],
                        start=True, stop=True,
                    )
                    if not POSTMASK:
                        # additive mask tiles for partial sub-blocks
                        for h, i in enumerate(sub_is):
                            if status[j, i] == PARTIAL:
                                uid = uid_of[(j, i)]
                                nc.any.tensor_add(
                                    sc[:, co + h * P : co + (h + 1) * P],
                                    sc[:, co + h * P : co + (h + 1) * P],
                                    mask_sb[:, uid * P : (uid + 1) * P],
                                )
                et = e_pool.tile([P, GROUP * CHUNK], F16, tag="expT")
                # exp over the live column intervals (merged across cells)
                cells = [lv for row in live for lv in row]  # group-tile order
                intervals = []
                for idx, lv in enumerate(cells[: width // P]):
                    if lv and intervals and intervals[-1][1] == idx * P:
                        intervals[-1] = (intervals[-1][0], (idx + 1) * P)
                    elif lv:
                        intervals.append((idx * P, (idx + 1) * P))
                if not TRIM:
                    intervals = [(0, width)]
                for lo, hi in intervals:
                    nc.scalar.activation(
                        et[:, lo:hi], sc[:, lo:hi],
                        mybir.ActivationFunctionType.Exp,
                        scale=inv_norm,
                    )
                if POSTMASK:
                    # zero masked entries after exp (0/1 fp16 mask tiles)
                    for k, j in enumerate(gjs):
                        co = k * CHUNK
                        for h, i in enumerate(
                            range(ci * CHUNK // P, (ci + 1) * CHUNK // P)
                        ):
                            if status[j, i] == PARTIAL:
                                uid = uid_of[(j, i)]
                                nc.any.tensor_mul(
                                    et[:, co + h * P : co + (h + 1) * P],
                                    et[:, co + h * P : co + (h + 1) * P],
                                    mask_sb[:, uid * P : (uid + 1) * P],
                                )
                for k, j in enumerate(gjs):
                    exp_tiles[j] = (et, k * CHUNK)

            if SWPIPE:
                pending_pv.append((ci, i_tiles, exp_tiles))
                if len(pending_pv) > SWPIPE:
                    emit_pv(*pending_pv.pop(0))
            else:
                emit_pv(ci, i_tiles, exp_tiles)
        for args in pending_pv:
            emit_pv(*args)

    for s0 in range(0, SLOTS_PER_CORE, INTERLEAVE):
        gens = [slot_gen(s) for s in range(s0, min(s0 + INTERLEAVE, SLOTS_PER_CORE))]
        while gens:
            gens = [g for g in gens if next(g, StopIteration) is not StopIteration]


def _build_program(schedules, n_mask_tiles, n_iters=1):
    """Build the SPMD bass program. schedules[slot] = (status, uid_of).

    n_iters > 1 wraps the whole forward pass in a hardware loop so one NEFF
    execution runs the kernel n_iters times back-to-back (for HW timing).
    """
    nc = bacc.Bacc()

    qT_d = nc.declare_dram_parameter("qT", [SLOTS_PER_CORE, P, SQ], F16, isOutput=False)
    kT_d = nc.declare_dram_parameter("kT", [SLOTS_PER_CORE, P, SK], F16, isOutput=False)
    v_d = nc.declare_dram_parameter(
        "v_aug", [SLOTS_PER_CORE, P, N_SK_TILES, HN + 1], F16, isOutput=False
    )
    mt_d = None
    if n_mask_tiles:
        mt_d = nc.declare_dram_parameter(
            "mask_tiles",
            [P, n_mask_tiles * P],
            F16 if POSTMASK else F32,
            isOutput=False,
        )
    out_d = nc.declare_dram_parameter(
        "out", [SLOTS_PER_CORE, N_SQ_TILES, P, HN], F16, isOutput=True
    )

    with tile.TileContext(nc) as tc, ExitStack() as ctx:
        qk_pool = ctx.enter_context(tc.tile_pool(name="qk", bufs=QK_BUFS))
        v_pool = ctx.enter_context(
            tc.tile_pool(name="v", bufs=max(2, INTERLEAVE + 1))
        )
        m_pool = ctx.enter_context(tc.tile_pool(name="m", bufs=1))
        e_pool = ctx.enter_context(tc.tile_pool(name="e", bufs=E_BUFS))
        o_pool = ctx.enter_context(tc.tile_pool(name="o", bufs=4))
        r_pool = ctx.enter_context(tc.tile_pool(name="r", bufs=4))
        sc_ps = ctx.enter_context(tc.tile_pool(name="sc", bufs=SC_BUFS, space="PSUM"))
        cx_ps = ctx.enter_context(tc.tile_pool(name="cx", bufs=CX_BUFS, space="PSUM"))

        mask_sb = None
        if n_mask_tiles:
            mask_sb = m_pool.tile(
                [P, n_mask_tiles * P], F16 if POSTMASK else F32, tag="mask"
            )
            nc.sync.dma_start(mask_sb[:], mt_d[:])

        pools = (qk_pool, v_pool, e_pool, o_pool, r_pool, sc_ps, cx_ps)
        args = (nc, tc, ctx, schedules, mask_sb, qT_d, kT_d, v_d, out_d, pools)
        if n_iters == 1:
            _build_body(*args)
        else:
            with tc.For_i(0, n_iters):
                _build_body(*args)

    nc.compile()
    return nc


_cache = {}


def _get_program(mask: np.ndarray, n_iters=1):
    key = (mask.tobytes(), n_iters)
    if key in _cache:
        return _cache[key]

    # schedules per batch; slots [0,1] -> b=0, [2,3] -> b=1 (same for all cores)
    scheds = []
    all_tiles: list[np.ndarray] = []
    tile_index: dict[bytes, int] = {}
    for b in range(B):
        status, uniq, uid_of = _block_schedule(np.asarray(mask[b, 0]))
        remap = {}
        for local_uid, t in enumerate(uniq):
            k = t.tobytes()
            if k not in tile_index:
                tile_index[k] = len(all_tiles)
                all_tiles.append(t)
            remap[local_uid] = tile_index[k]
        uid_of = {ji: remap[u] for ji, u in uid_of.items()}
        scheds.append((status, uid_of))

    slot_scheds = [scheds[0], scheds[0], scheds[1], scheds[1]]
    n_tiles = len(all_tiles)
    nc = _build_program(slot_scheds, n_tiles, n_iters=n_iters)

    if n_tiles:
        mt = np.stack(all_tiles)  # [U, 128, 128] additive (0 / NEG)
        if POSTMASK:
            mt = (mt == 0.0).astype(np.float16)  # multiplicative 1/0
        mask_tiles = np.ascontiguousarray(mt.transpose(1, 0, 2)).reshape(
            P, n_tiles * P
        )
    else:
        mask_tiles = None
    _cache[key] = (nc, mask_tiles)
    return _cache[key]


def _core_slots(c):
    return [(0, 2 * c), (0, 2 * c + 1), (1, 2 * c), (1, 2 * c + 1)]


def prepare(query_layer, key_layer, value_layer, attention_mask, n_iters=None):
    """Build (nc, in_maps). Shared by kernel() and the benchmark harness."""
    if n_iters is None:
        n_iters = N_ITERS
    q = np.asarray(query_layer, dtype=np.float32)
    k = np.asarray(key_layer, dtype=np.float32)
    v = np.asarray(value_layer, dtype=np.float32)
    mask = np.asarray(attention_mask)

    nc, mask_tiles = _get_program(mask, n_iters)

    # host layout prep (cast to fp16 for half the HBM traffic; accumulations
    # on device stay fp32 in PSUM)
    # qT_all[b, n] = q[:, b, n, :].T  -> [B, NP, 128, SQ]
    qT_all = np.ascontiguousarray(q.transpose(1, 2, 3, 0), dtype=np.float16)
    kT_all = np.ascontiguousarray(k.transpose(1, 2, 3, 0), dtype=np.float16)
    # v_aug_all[b, n, p, t, c] = v[t*128+p, b, n, c], plus ones column
    v5 = v.reshape(N_SK_TILES, P, B, NP, HN).transpose(2, 3, 1, 0, 4)
    v_aug_all = np.empty((B, NP, P, N_SK_TILES, HN + 1), dtype=np.float16)
    v_aug_all[..., :HN] = v5
    v_aug_all[..., HN] = 1.0

    in_maps = []
    for c in range(N_CORES):
        slots = _core_slots(c)
        im = {
            "qT": np.ascontiguousarray(np.stack([qT_all[b, n] for b, n in slots])),
            "kT": np.ascontiguousarray(np.stack([kT_all[b, n] for b, n in slots])),
            "v_aug": np.ascontiguousarray(
                np.stack([v_aug_all[b, n] for b, n in slots])
            ),
        }
        if mask_tiles is not None:
            im["mask_tiles"] = mask_tiles
        in_maps.append(im)
    return nc, in_maps


def assemble(results):
    """Gather per-core 'out' arrays into the full [SQ, B, NP*HN] output."""
    full = np.empty((SQ, B, NP * HN), dtype=np.float32)
    for c in range(N_CORES):
        o = results[c]["out"]  # [4, 16, 128, 128] fp16
        for s, (b, n) in enumerate(_core_slots(c)):
            full[:, b, n * HN : (n + 1) * HN] = o[s].reshape(SQ, HN).astype(
                np.float32
            )
    return full


def kernel(query_layer, key_layer, value_layer, attention_mask):
    from concourse.bass_utils import run_bass_kernel_spmd

    nc, in_maps = prepare(
        query_layer, key_layer, value_layer, attention_mask, n_iters=1
    )
    res = run_bass_kernel_spmd(nc, in_maps, list(range(N_CORES)))
    return assemble(res.results)
